# revision 33
# baseline (speedup 1.0000x reference)
"""Multi-head self-attention TRN2 Bass kernel v7 (8 NeuronCores).

Sharding: core c -> batch b = c//2, head-group g = c%2 (8 of 16 heads).
Host sums the two partial output projections per batch.

v7 vs v6:
  - wq/wk/wv shipped host-transposed so weight DMAs are contiguous
    (256B descriptor lines -> 2KB), shrinking the preamble stall.
  - Output projection split j=0,1 (into bf16 y01 accumulators, run in
    hp1/hp2 exp-idle slots) + j=2,3 (+add) in hp3 / tail, so hp3 blocks
    drop to the exp floor and the tail only carries 16 matmuls.
  - hp3 block boundaries get post-block projection units (the exp
    stream drains ~2 chunks past each block end; PE needs queued work).
  - Final carry normalize pipelined per head-half across DVE/Pool,
    reading PV straight from PSUM (saves ~2.5us on the tail).
  - Tail adds alternate DVE and scalar-copy+Pool; output staged in a
    dedicated SBUF tile (no buffer-reuse waits on DMA completion).
"""
import numpy as np
from contextlib import ExitStack

import concourse.bass as bass
import concourse.mybir as mybir
import concourse.tile as tile
from concourse import bacc
from concourse.bass_utils import run_bass_kernel_spmd
import ml_dtypes

f32, f32r, bf16 = mybir.dt.float32, mybir.dt.float32r, mybir.dt.bfloat16
EXP = mybir.ActivationFunctionType.Exp
CPY = mybir.ActivationFunctionType.Copy
MULT = mybir.AluOpType.mult
ADD = mybir.AluOpType.add

B, N = 4, 2048
DIM = 1024
HL = 8
DH = 64
KD = DIM // 128
HP = HL // 2


def build(SEQ=2048):
    NS = SEQ // 128
    NQ = SEQ // 512

    nc = bacc.Bacc(None, target_bir_lowering=False, debug=False)
    xT = nc.declare_dram_parameter("xT", [DIM, SEQ], bf16, isOutput=False)
    wq = nc.declare_dram_parameter("wq", [128, HP * KD * 128], bf16,
                                   isOutput=False)
    wk = nc.declare_dram_parameter("wk", [128, HP * KD * 128], bf16,
                                   isOutput=False)
    wv = nc.declare_dram_parameter("wv", [128, KD * 512], bf16, isOutput=False)
    wo = nc.declare_dram_parameter("wo", [HL * DH, DIM], bf16, isOutput=False)
    yT = nc.declare_dram_parameter("yT", [DIM, SEQ], bf16, isOutput=True)

    with tile.TileContext(nc) as tc, ExitStack() as ctx:
        p1 = ctx.enter_context(tc.tile_pool(name="p1", bufs=1))
        p_pt = ctx.enter_context(tc.tile_pool(name="p_pt", bufs=6))
        p_w = ctx.enter_context(tc.tile_pool(name="p_w", bufs=2))
        p_y = ctx.enter_context(tc.tile_pool(name="p_y", bufs=4))
        p_ev = ctx.enter_context(tc.tile_pool(name="p_ev", bufs=2))
        psS = ctx.enter_context(tc.tile_pool(name="psS", bufs=2, space="PSUM"))
        psPV = ctx.enter_context(tc.tile_pool(name="psPV", bufs=1, space="PSUM"))
        psA = ctx.enter_context(tc.tile_pool(name="psA", bufs=2, space="PSUM"))

        # ---- persistent SBUF tiles -------------------------------------
        xt_all = p1.tile([128, KD * SEQ], bf16, tag="xt", name="xt")
        xt = [xt_all[:, k * SEQ:(k + 1) * SEQ] for k in range(KD)]
        wvt = p1.tile([128, KD * 512], bf16, tag="wvt", name="wvt")
        wot = p1.tile([128, HP * DIM], bf16, tag="wot", name="wot")
        q2 = [p1.tile([128, SEQ], bf16, tag=f"q2_{i}", name=f"q2_{i}")
              for i in range(2)]
        k2 = [p1.tile([128, SEQ], bf16, tag=f"k2_{i}", name=f"k2_{i}")
              for i in range(2)]
        v2 = [p1.tile([128, HL * 65 + 64], bf16, tag=f"v2_{st}", name=f"v2_{st}")
              for st in range(NS)]
        ot = [p1.tile([128, SEQ], bf16, tag=f"ot{j}", name=f"ot{j}")
              for j in range(HP)]
        # j=0,1 partial projections, one per query block (bf16)
        y01 = [p1.tile([128, KD * 512], bf16, tag=f"y01_{qb}", name=f"y01_{qb}")
               for qb in range(NQ)]
        yfin = p1.tile([128, KD * 512], bf16, tag="yfin", name="yfin")

        # ---- HAM warmup: dummy matmuls independent of any DMA ----------
        wsc = p1.tile([128, 512], bf16, tag="wsc", name="wsc")
        nc.gpsimd.memset(wsc[:], 0.0)
        for i in range(2):
            wps = psA.tile([128, 512], f32, tag="aux", name=f"hw{i}")
            for r in range(10):
                nc.tensor.matmul(wps[:], wsc[:, 0:128], wsc[:],
                                 start=(r == 0), stop=(r == 9))

        def warm(n, pool=None):
            """Discardable matmuls bridging PE over exp-stream restarts."""
            if pool is None:
                wps = psA.tile([128, 512], f32, tag="aux", name="warm")
            else:
                wps = pool.tile([128, 1024], f32, tag="s", name="warms")
            for r in range(n):
                nc.tensor.matmul(wps[:, 0:512], wsc[:, 0:128], wsc[:],
                                 start=(r == 0), stop=(r == n - 1))

        # ---- DMA issue (contiguous descriptors, consumption order) -----
        wt_q, wt_k = {}, {}

        def dma_qk_weights(hp):
            for which, wsrc, store in (("q", wq, wt_q), ("k", wk, wt_k)):
                t = p_w.tile([128, KD * 128], bf16, tag=f"w{which}",
                             name=f"w{which}{hp}")
                nc.sync.dma_start(
                    out=t[:], in_=wsrc[:, hp * KD * 128:(hp + 1) * KD * 128])
                store[hp] = t

        # hp0 weights split around x quarter 0: q-unit 0 needs wq+x only
        tq = p_w.tile([128, KD * 128], bf16, tag="wq", name="wq0")
        nc.sync.dma_start(out=tq[:], in_=wq[:, 0:KD * 128])
        wt_q[0] = tq
        for quarter in range(4):
            qsl2 = slice(quarter * (SEQ // 4), (quarter + 1) * (SEQ // 4))
            nc.sync.dma_start(
                out=xt_all[:].rearrange("p (k c) -> p k c", k=KD)[:, :, qsl2],
                in_=xT[:].rearrange("(k p) c -> p k c", k=KD)[:, :, qsl2])
            if quarter == 0:
                tk = p_w.tile([128, KD * 128], bf16, tag="wk", name="wk0")
                nc.sync.dma_start(out=tk[:], in_=wk[:, 0:KD * 128])
                wt_k[0] = tk
                nc.sync.dma_start(out=wvt[:], in_=wv[:])
        nc.sync.dma_start(
            out=wot[:].rearrange("p (j c) -> p j c", j=HP),
            in_=wo[:].rearrange("(j p) c -> p j c", j=HP))

        # ---- phase helpers ---------------------------------------------
        def qk_unit(hp, which, u):
            dst = (q2 if which == "q" else k2)[hp % 2]
            wt = (wt_q if which == "q" else wt_k)[hp]
            usl = slice(u * 512, (u + 1) * 512)
            pss = psA.tile([128, 512], f32, tag="aux", name=f"qk{which}{u}")
            for k in range(KD):
                nc.tensor.matmul(pss[:], wt[:, k * 128:(k + 1) * 128],
                                 xt[k][:, usl],
                                 start=(k == 0), stop=(k == KD - 1))
            nc.vector.tensor_copy(out=dst[:, usl], in_=pss[:])

        def v_unit(st):
            v2t = v2[st]
            v3 = v2t[:, 0:HL * 65].rearrange("p (h c) -> p h c", h=HL)
            nc.vector.memset(v3[:, :, 64:65], 1.0)
            nc.vector.memset(v2t[:, HL * 65:], 0.0)
            vps = psA.tile([128, HL * DH], f32, tag="aux", name=f"v{st}")
            for k in range(KD):
                nc.tensor.matmul(vps[:], xt[k][:, st * 128:(st + 1) * 128],
                                 wvt[:, k * 512:(k + 1) * 512],
                                 start=(k == 0), stop=(k == KD - 1))
            nc.vector.tensor_copy(
                out=v3[:, :, 0:64],
                in_=vps[:].rearrange("p (h d) -> p h d", h=HL))

        def emit_scores(hp, qb, c):
            """Score matmuls + exp for one key chunk; returns the pt tile."""
            q2t, k2t = q2[hp % 2], k2[hp % 2]
            qsl = slice(qb * 512, (qb + 1) * 512)
            csl = slice(c * 128, (c + 1) * 128)
            s = psS.tile([128, 1024], f32, tag="s", name="s")
            nc.tensor.matmul(s[:, 0:512], k2t[0:64, csl], q2t[0:64, qsl],
                             start=True, stop=True)
            nc.tensor.matmul(s[:, 512:1024], k2t[64:128, csl],
                             q2t[64:128, qsl], start=True, stop=True)
            pt = p_pt.tile([128, 1024], bf16, tag="pt", name="pt")
            nc.scalar.activation(pt[:], s[:], EXP, scale=0.125)
            return pt

        def attention(hp, qb, filler=None, carry_in=None, final=False,
                      pre=None, preview_fn=None):
            """Software-pipelined: PV lags its chunk by 4 slots; the
            previous block's last PVs + evict (carry_in) land at slot 1.
            Emission order per chunk: scores -> exp -> PV -> fillers.
            `pre` is this block's chunk-0 pt if the previous block
            previewed it; `preview_fn` emits the next block's chunk-0
            scores+exp before this block's trailing PVs so the exp
            stream never idles across the boundary."""
            qsl = slice(qb * 512, (qb + 1) * 512)
            l0, l1 = hp * 2, hp * 2 + 1
            pv = psPV.tile([128, 1024], f32, tag="pv", name="pv")
            pts = {}

            def emit_pv(c):
                pt = pts.pop(c)
                nc.tensor.matmul(pv[:, 0:512], v2[c][:, l0 * 65:l0 * 65 + 128],
                                 pt[:, 0:512],
                                 start=(c == 0), stop=(c == NS - 1))
                nc.tensor.matmul(pv[:, 512:1024],
                                 v2[c][:, l1 * 65:l1 * 65 + 128],
                                 pt[:, 512:1024],
                                 start=(c == 0), stop=(c == NS - 1))

            for c in range(NS):
                if c == 0 and pre is not None:
                    pts[0] = pre
                else:
                    pts[c] = emit_scores(hp, qb, c)
                if c == 1 and carry_in is not None:
                    carry_in()
                if c >= 4:
                    emit_pv(c - 4)
                if filler is not None:
                    for fn in filler.get(c, ()):
                        fn()
            nxt_pt = preview_fn() if preview_fn is not None else None
            emit_pv(NS - 4)
            emit_pv(NS - 3)

            def carry_out():
                emit_pv(NS - 2)
                emit_pv(NS - 1)
                # fast-evict pv to SBUF, normalize off the critical path
                pvs = p_ev.tile([64, 1024], f32, tag="pvs", name="pvs")
                nc.vector.tensor_copy(out=pvs[:], in_=pv[0:64, :])
                srow = p1.tile([1, 1024], f32, tag="srow", name="srow")
                nc.vector.tensor_copy(out=srow[:], in_=pv[64:65, :])
                rsum = p1.tile([1, 1024], f32, tag="rsum", name="rsum")
                nc.vector.reciprocal_approx_fast(out=rsum[:], in_=srow[:])
                for h01 in range(2):
                    hsl = slice(h01 * 512, (h01 + 1) * 512)
                    rb = p1.tile([64, 512], f32, tag=f"rb{h01}", name="rb")
                    nc.gpsimd.partition_broadcast(rb[:], rsum[:, hsl])
                    lo = h01 * 64
                    nc.vector.tensor_tensor(out=ot[hp][lo:lo + 64, qsl],
                                            in0=pvs[0:64, hsl], in1=rb[:],
                                            op=MULT)

            def carry_final():
                emit_pv(NS - 2)
                emit_pv(NS - 1)
                # pipelined per-half: DVE row-copies first, Pool
                # broadcasts overlap, recip+mult read PV from PSUM
                srow = p1.tile([1, 1024], f32, tag="srow", name="srowf")
                for h01 in range(2):
                    hsl = slice(h01 * 512, (h01 + 1) * 512)
                    nc.vector.tensor_copy(out=srow[:, hsl],
                                          in_=pv[64:65, hsl])
                rbs = []
                for h01 in range(2):
                    hsl = slice(h01 * 512, (h01 + 1) * 512)
                    rbh = p1.tile([64, 512], f32, tag=f"rb{h01}", name="rbf")
                    nc.gpsimd.partition_broadcast(rbh[:], srow[:, hsl])
                    rbs.append(rbh)
                for h01 in range(2):
                    hsl = slice(h01 * 512, (h01 + 1) * 512)
                    nc.vector.reciprocal_approx_fast(out=rbs[h01][:],
                                                     in_=rbs[h01][:])
                    lo = h01 * 64
                    nc.vector.tensor_tensor(out=ot[hp][lo:lo + 64, qsl],
                                            in0=pv[0:64, hsl], in1=rbs[h01][:],
                                            op=MULT)

            return (carry_final if final else carry_out), nxt_pt

        def o01(qb, dt):
            """j=0,1 half of the output projection -> bf16 y01."""
            nsl = slice(qb * 512, (qb + 1) * 512)
            yps = psA.tile([128, 512], f32, tag="aux", name=f"o01_{dt}")
            for j in range(2):
                nc.tensor.matmul(
                    yps[:], wot[:, j * DIM + dt * 128:j * DIM + (dt + 1) * 128],
                    ot[j][:, nsl], start=(j == 0), stop=(j == 1))
            nc.vector.tensor_copy(out=y01[qb][:, dt * 512:(dt + 1) * 512],
                                  in_=yps[:])

        def o23(qb, dt):
            """j=2,3 half + add y01 + store."""
            nsl = slice(qb * 512, (qb + 1) * 512)
            yps = psA.tile([128, 512], f32, tag="aux", name=f"o23_{dt}")
            for j in (2, 3):
                nc.tensor.matmul(
                    yps[:], wot[:, j * DIM + dt * 128:j * DIM + (dt + 1) * 128],
                    ot[j][:, nsl], start=(j == 2), stop=(j == 3))
            ysb = p_y.tile([128, 512], bf16, tag="y", name="ysb")
            nc.vector.tensor_tensor(out=ysb[:], in0=yps[:],
                                    in1=y01[qb][:, dt * 512:(dt + 1) * 512],
                                    op=ADD)
            nc.sync.dma_start(out=yT[dt * 128:(dt + 1) * 128, nsl],
                              in_=ysb[:])

        def o23_tail(qb, dt):
            """Tail variant: stage into yfin (no buffer-reuse waits);
            odd dt evicts via Scalar + adds on Pool to unload DVE."""
            nsl = slice(qb * 512, (qb + 1) * 512)
            dsl = slice(dt * 512, (dt + 1) * 512)
            yps = psA.tile([128, 512], f32, tag="aux", name=f"of{dt}")
            for j in (2, 3):
                nc.tensor.matmul(
                    yps[:], wot[:, j * DIM + dt * 128:j * DIM + (dt + 1) * 128],
                    ot[j][:, nsl], start=(j == 2), stop=(j == 3))
            if dt % 2 == 0:
                nc.vector.tensor_tensor(out=yfin[:, dsl], in0=yps[:],
                                        in1=y01[qb][:, dsl], op=ADD)
            else:
                ycp = p_y.tile([128, 512], bf16, tag="y", name="ycp")
                nc.scalar.activation(ycp[:], yps[:], CPY)
                nc.gpsimd.tensor_tensor(out=yfin[:, dsl], in0=ycp[:],
                                        in1=y01[qb][:, dsl], op=ADD)
            nc.sync.dma_start(out=yT[dt * 128:(dt + 1) * 128, nsl],
                              in_=yfin[:, dsl])

        # ---- emission schedule -----------------------------------------
        qk_unit(0, "q", 0)
        qk_unit(0, "k", 0)

        V = lambda st: (lambda: v_unit(st))
        QK = lambda which, u: (lambda: qk_unit(0, which, u))
        O01 = lambda qb, dt: (lambda: o01(qb, dt))
        O23 = lambda qb, dt: (lambda: o23(qb, dt))
        fill_qb0 = {
            0: [V(0)], 1: [V(1), V(2)], 2: [V(3), QK("k", 1)], 3: [V(4)],
            4: [V(5)], 5: [V(6), QK("k", 2)], 6: [V(7)], 7: [V(8)],
            8: [V(9)], 9: [V(10), QK("k", 3)], 10: [V(11)],
            11: [V(12), QK("q", 1)], 12: [V(13)], 13: [V(14)], 14: [V(15)],
        }

        QKU = lambda hp, which, u: (lambda: qk_unit(hp, which, u))
        W2 = lambda: (lambda: warm(2))
        # filler schedule per (hp, qb): {slot: [thunks]}. Slots 0-3 are
        # the block-start hole (no PV matmuls until slot 4); next-hp
        # qk_units go there (slots 1/3) instead of trailing the block,
        # so the next block's scores reach the exp stream immediately.
        fills = {
            (0, 0): fill_qb0,
            (0, 1): {1: [QKU(1, "q", 0)], 2: [QK("q", 2)],
                     3: [QKU(1, "k", 0)]},
            (0, 2): {1: [QKU(1, "q", 1)], 2: [QK("q", 3)],
                     3: [QKU(1, "k", 1)]},
            (0, 3): {1: [QKU(1, "q", 2)], 3: [QKU(1, "k", 2)]},
            (1, 0): {1: [QKU(1, "q", 3)], 3: [QKU(1, "k", 3)]},
            (1, 1): {1: [QKU(2, "q", 0)], 3: [QKU(2, "k", 0)],
                     6: [O01(0, 0)], 8: [O01(0, 1)]},
            (1, 2): {1: [QKU(2, "q", 1)], 3: [QKU(2, "k", 1)],
                     5: [O01(0, 2)], 7: [O01(0, 3)], 9: [O01(0, 4)]},
            (1, 3): {1: [QKU(2, "q", 2)], 3: [QKU(2, "k", 2)],
                     5: [O01(0, 5)], 7: [O01(0, 6)], 9: [O01(0, 7)]},
            (2, 0): {1: [QKU(2, "q", 3)], 3: [QKU(2, "k", 3)],
                     5: [O01(1, 0)], 7: [O01(1, 1)], 9: [O01(1, 2)]},
            (2, 1): {1: [QKU(3, "q", 0)], 3: [QKU(3, "k", 0)],
                     5: [O01(1, 3)], 7: [O01(1, 4)], 9: [O01(1, 5)]},
            (2, 2): {1: [QKU(3, "q", 1)], 3: [QKU(3, "k", 1)],
                     5: [O01(1, 6)], 7: [O01(1, 7)], 9: [O01(2, 0)]},
            (2, 3): {1: [QKU(3, "q", 2)], 3: [QKU(3, "k", 2)],
                     5: [O01(2, 1)], 7: [O01(2, 2)], 9: [O01(2, 3)]},
            (3, 0): {1: [QKU(3, "q", 3)], 3: [QKU(3, "k", 3)],
                     5: [O01(2, 4)], 6: [O01(2, 5)], 7: [O01(2, 6)],
                     8: [O01(2, 7)], 9: [O01(3, 0)], 10: [O01(3, 1)],
                     11: [O01(3, 2)], 12: [O01(3, 3)]},
            (3, 1): {0: [W2()], 1: [W2()], 3: [W2()],
                     5: [O01(3, 4)], 6: [O01(3, 5)], 7: [O01(3, 6)],
                     8: [O01(3, 7)], 9: [O23(0, 0)], 10: [O23(0, 1)],
                     11: [O23(0, 2)], 12: [O23(0, 3)]},
            (3, 2): {0: [W2()], 1: [W2()], 3: [W2()],
                     5: [O23(0, 4)], 6: [O23(0, 5)], 7: [O23(0, 6)],
                     8: [O23(0, 7)], 9: [O23(1, 0)], 10: [O23(1, 1)],
                     11: [O23(1, 2)], 12: [O23(1, 3)]},
            (3, 3): {0: [W2()], 1: [W2()], 3: [W2()],
                     5: [O23(1, 4)], 6: [O23(1, 5)], 7: [O23(1, 6)],
                     8: [O23(1, 7)], 9: [O23(2, 0)], 10: [O23(2, 1)],
                     11: [O23(2, 2)], 12: [O23(2, 3)], 13: [O23(2, 4)]},
        }

        blocks = [(hp, qb) for hp in range(HP) for qb in range(NQ)]
        carry = None
        pre = None
        for i, (hp, qb) in enumerate(blocks):
            if qb == 0 and hp + 1 < HP:
                dma_qk_weights(hp + 1)
            final = (i == len(blocks) - 1)
            if final:
                preview_fn = None
            else:
                nhp, nqb = blocks[i + 1]
                preview_fn = (lambda h, q: (lambda: emit_scores(h, q, 0)))(
                    nhp, nqb)
            carry, pre = attention(hp, qb, fills.get((hp, qb)), carry,
                                   final=final, pre=pre,
                                   preview_fn=preview_fn)
        carry()
        # bridge PE over the final normalize: qb2's last o23 units hold
        # the two psA slots exactly until ot[3] is ready (their adds sit
        # behind the normalize on DVE), warms on freed psS cover the rest
        o23(NQ - 2, 5)
        o23(NQ - 2, 6)
        warm(6, pool=psS)
        warm(6, pool=psS)
        o23(NQ - 2, 7)
        for dt in range(KD):
            o23_tail(NQ - 1, dt)

    nc.finalize()
    return nc


def make_in_map(x_b, w_qkv, w_out, g):
    cols = slice(g * 512, (g + 1) * 512)
    b = ml_dtypes.bfloat16

    def qk_layout(w):
        # device reads [128, hp, k, 128] with contiguous per-hp slices
        return np.ascontiguousarray(
            w.reshape(KD, 128, HP, 128).transpose(1, 2, 0, 3)
             .reshape(128, HP * KD * 128).astype(b))

    wv_s = w_qkv[:, 2048:3072][:, cols]
    return {
        "xT": np.ascontiguousarray(x_b.T.astype(b)),
        "wq": qk_layout(w_qkv[:, 0:1024][:, cols]),
        "wk": qk_layout(w_qkv[:, 1024:2048][:, cols]),
        "wv": np.ascontiguousarray(
            wv_s.reshape(KD, 128, 512).transpose(1, 0, 2)
                .reshape(128, KD * 512).astype(b)),
        "wo": np.ascontiguousarray(w_out[cols, :].astype(b)),
    }


_NC_CACHE = {}


def _get_nc():
    if "nc" not in _NC_CACHE:
        _NC_CACHE["nc"] = build()
    return _NC_CACHE["nc"]


def kernel(x, w_qkv, w_out, b_out, trace=False):
    x = np.ascontiguousarray(np.asarray(x, dtype=np.float32))
    w_qkv = np.ascontiguousarray(np.asarray(w_qkv, dtype=np.float32))
    w_out = np.ascontiguousarray(np.asarray(w_out, dtype=np.float32))
    b_out = np.asarray(b_out, dtype=np.float32)

    nc = _get_nc()
    in_maps = [make_in_map(x[c // 2], w_qkv, w_out, c % 2) for c in range(8)]
    r = run_bass_kernel_spmd(nc, in_maps, list(range(8)), trace=trace)
    _NC_CACHE["exec_time_ns"] = r.exec_time_ns

    out = np.empty((B, N, DIM), np.float32)
    for b in range(B):
        out[b] = (r.results[2 * b]["yT"].astype(np.float32)
                  + r.results[2 * b + 1]["yT"].astype(np.float32)).T + b_out
    return out


# revision 35
# speedup vs baseline: 1.0038x; 1.0038x over previous
"""Multi-head self-attention TRN2 Bass kernel v7 (8 NeuronCores).

Sharding: core c -> batch b = c//2, head-group g = c%2 (8 of 16 heads).
Host sums the two partial output projections per batch.

v7 vs v6:
  - wq/wk/wv shipped host-transposed so weight DMAs are contiguous
    (256B descriptor lines -> 2KB), shrinking the preamble stall.
  - Output projection split j=0,1 (into bf16 y01 accumulators, run in
    hp1/hp2 exp-idle slots) + j=2,3 (+add) in hp3 / tail, so hp3 blocks
    drop to the exp floor and the tail only carries 16 matmuls.
  - hp3 block boundaries get post-block projection units (the exp
    stream drains ~2 chunks past each block end; PE needs queued work).
  - Final carry normalize pipelined per head-half across DVE/Pool,
    reading PV straight from PSUM (saves ~2.5us on the tail).
  - Tail adds alternate DVE and scalar-copy+Pool; output staged in a
    dedicated SBUF tile (no buffer-reuse waits on DMA completion).
"""
import numpy as np
from contextlib import ExitStack

import concourse.bass as bass
import concourse.mybir as mybir
import concourse.tile as tile
from concourse import bacc
from concourse.bass_utils import run_bass_kernel_spmd
import ml_dtypes

f32, f32r, bf16 = mybir.dt.float32, mybir.dt.float32r, mybir.dt.bfloat16
EXP = mybir.ActivationFunctionType.Exp
CPY = mybir.ActivationFunctionType.Copy
MULT = mybir.AluOpType.mult
ADD = mybir.AluOpType.add

B, N = 4, 2048
DIM = 1024
HL = 8
DH = 64
KD = DIM // 128
HP = HL // 2


def build(SEQ=2048):
    NS = SEQ // 128
    NQ = SEQ // 512

    nc = bacc.Bacc(None, target_bir_lowering=False, debug=False)
    xT = nc.declare_dram_parameter("xT", [DIM, SEQ], bf16, isOutput=False)
    wq = nc.declare_dram_parameter("wq", [128, HP * KD * 128], bf16,
                                   isOutput=False)
    wk = nc.declare_dram_parameter("wk", [128, HP * KD * 128], bf16,
                                   isOutput=False)
    wv = nc.declare_dram_parameter("wv", [128, KD * 512], bf16, isOutput=False)
    wo = nc.declare_dram_parameter("wo", [HL * DH, DIM], bf16, isOutput=False)
    yT = nc.declare_dram_parameter("yT", [DIM, SEQ], bf16, isOutput=True)

    with tile.TileContext(nc) as tc, ExitStack() as ctx:
        p1 = ctx.enter_context(tc.tile_pool(name="p1", bufs=1))
        p_pt = ctx.enter_context(tc.tile_pool(name="p_pt", bufs=6))
        p_w = ctx.enter_context(tc.tile_pool(name="p_w", bufs=2))
        p_y = ctx.enter_context(tc.tile_pool(name="p_y", bufs=4))
        p_ev = ctx.enter_context(tc.tile_pool(name="p_ev", bufs=2))
        psS = ctx.enter_context(tc.tile_pool(name="psS", bufs=2, space="PSUM"))
        psPV = ctx.enter_context(tc.tile_pool(name="psPV", bufs=1, space="PSUM"))
        psA = ctx.enter_context(tc.tile_pool(name="psA", bufs=2, space="PSUM"))

        # ---- persistent SBUF tiles -------------------------------------
        xt_all = p1.tile([128, KD * SEQ], bf16, tag="xt", name="xt")
        xt = [xt_all[:, k * SEQ:(k + 1) * SEQ] for k in range(KD)]
        wvt = p1.tile([128, KD * 512], bf16, tag="wvt", name="wvt")
        wot = p1.tile([128, HP * DIM], bf16, tag="wot", name="wot")
        q2 = [p1.tile([128, SEQ], bf16, tag=f"q2_{i}", name=f"q2_{i}")
              for i in range(2)]
        k2 = [p1.tile([128, SEQ], bf16, tag=f"k2_{i}", name=f"k2_{i}")
              for i in range(2)]
        v2 = [p1.tile([128, HL * 65 + 64], bf16, tag=f"v2_{st}", name=f"v2_{st}")
              for st in range(NS)]
        ot = [p1.tile([128, SEQ], bf16, tag=f"ot{j}", name=f"ot{j}")
              for j in range(HP)]
        # j=0,1 partial projections, one per query block (bf16)
        y01 = [p1.tile([128, KD * 512], bf16, tag=f"y01_{qb}", name=f"y01_{qb}")
               for qb in range(NQ)]
        yfin = p1.tile([128, KD * 512], bf16, tag="yfin", name="yfin")

        # ---- HAM warmup: dummy matmuls independent of any DMA ----------
        wsc = p1.tile([128, 512], bf16, tag="wsc", name="wsc")
        nc.gpsimd.memset(wsc[:], 0.0)
        for i in range(2):
            wps = psA.tile([128, 512], f32, tag="aux", name=f"hw{i}")
            for r in range(10):
                nc.tensor.matmul(wps[:], wsc[:, 0:128], wsc[:],
                                 start=(r == 0), stop=(r == 9))

        def warm(n, pool=None):
            """Discardable matmuls bridging PE over exp-stream restarts."""
            if pool is None:
                wps = psA.tile([128, 512], f32, tag="aux", name="warm")
            else:
                wps = pool.tile([128, 1024], f32, tag="s", name="warms")
            for r in range(n):
                nc.tensor.matmul(wps[:, 0:512], wsc[:, 0:128], wsc[:],
                                 start=(r == 0), stop=(r == n - 1))

        # ---- DMA issue (contiguous descriptors, consumption order) -----
        wt_q, wt_k = {}, {}

        def dma_qk_weights(hp):
            for which, wsrc, store in (("q", wq, wt_q), ("k", wk, wt_k)):
                t = p_w.tile([128, KD * 128], bf16, tag=f"w{which}",
                             name=f"w{which}{hp}")
                nc.sync.dma_start(
                    out=t[:], in_=wsrc[:, hp * KD * 128:(hp + 1) * KD * 128])
                store[hp] = t

        # hp0 weights split around x quarter 0: q-unit 0 needs wq+x only
        tq = p_w.tile([128, KD * 128], bf16, tag="wq", name="wq0")
        nc.sync.dma_start(out=tq[:], in_=wq[:, 0:KD * 128])
        wt_q[0] = tq
        for quarter in range(4):
            qsl2 = slice(quarter * (SEQ // 4), (quarter + 1) * (SEQ // 4))
            nc.sync.dma_start(
                out=xt_all[:].rearrange("p (k c) -> p k c", k=KD)[:, :, qsl2],
                in_=xT[:].rearrange("(k p) c -> p k c", k=KD)[:, :, qsl2])
            if quarter == 0:
                tk = p_w.tile([128, KD * 128], bf16, tag="wk", name="wk0")
                nc.sync.dma_start(out=tk[:], in_=wk[:, 0:KD * 128])
                wt_k[0] = tk
                nc.sync.dma_start(out=wvt[:], in_=wv[:])
        nc.sync.dma_start(
            out=wot[:].rearrange("p (j c) -> p j c", j=HP),
            in_=wo[:].rearrange("(j p) c -> p j c", j=HP))

        # ---- phase helpers ---------------------------------------------
        def qk_unit(hp, which, u):
            dst = (q2 if which == "q" else k2)[hp % 2]
            wt = (wt_q if which == "q" else wt_k)[hp]
            usl = slice(u * 512, (u + 1) * 512)
            pss = psA.tile([128, 512], f32, tag="aux", name=f"qk{which}{u}")
            for k in range(KD):
                nc.tensor.matmul(pss[:], wt[:, k * 128:(k + 1) * 128],
                                 xt[k][:, usl],
                                 start=(k == 0), stop=(k == KD - 1))
            nc.vector.tensor_copy(out=dst[:, usl], in_=pss[:])

        def v_unit(st):
            v2t = v2[st]
            v3 = v2t[:, 0:HL * 65].rearrange("p (h c) -> p h c", h=HL)
            nc.vector.memset(v3[:, :, 64:65], 1.0)
            nc.vector.memset(v2t[:, HL * 65:], 0.0)
            vps = psA.tile([128, HL * DH], f32, tag="aux", name=f"v{st}")
            for k in range(KD):
                nc.tensor.matmul(vps[:], xt[k][:, st * 128:(st + 1) * 128],
                                 wvt[:, k * 512:(k + 1) * 512],
                                 start=(k == 0), stop=(k == KD - 1))
            nc.vector.tensor_copy(
                out=v3[:, :, 0:64],
                in_=vps[:].rearrange("p (h d) -> p h d", h=HL))

        def emit_scores(hp, qb, c):
            """Score matmuls + exp for one key chunk; returns the pt tile."""
            q2t, k2t = q2[hp % 2], k2[hp % 2]
            qsl = slice(qb * 512, (qb + 1) * 512)
            csl = slice(c * 128, (c + 1) * 128)
            s = psS.tile([128, 1024], f32, tag="s", name="s")
            nc.tensor.matmul(s[:, 0:512], k2t[0:64, csl], q2t[0:64, qsl],
                             start=True, stop=True)
            nc.tensor.matmul(s[:, 512:1024], k2t[64:128, csl],
                             q2t[64:128, qsl], start=True, stop=True)
            pt = p_pt.tile([128, 1024], bf16, tag="pt", name="pt")
            nc.scalar.activation(pt[:], s[:], EXP, scale=0.125)
            return pt

        def attention(hp, qb, filler=None, carry_in=None, final=False,
                      pre=None, preview_fn=None):
            """Software-pipelined: PV lags its chunk by 4 slots; the
            previous block's last PVs + evict (carry_in) land at slot 1.
            Emission order per chunk: scores -> exp -> PV -> fillers.
            `pre` is this block's chunk-0 pt if the previous block
            previewed it; `preview_fn` emits the next block's chunk-0
            scores+exp before this block's trailing PVs so the exp
            stream never idles across the boundary."""
            qsl = slice(qb * 512, (qb + 1) * 512)
            l0, l1 = hp * 2, hp * 2 + 1
            pv = psPV.tile([128, 1024], f32, tag="pv", name="pv")
            pts = {}

            def emit_pv(c):
                pt = pts.pop(c)
                nc.tensor.matmul(pv[:, 0:512], v2[c][:, l0 * 65:l0 * 65 + 128],
                                 pt[:, 0:512],
                                 start=(c == 0), stop=(c == NS - 1))
                nc.tensor.matmul(pv[:, 512:1024],
                                 v2[c][:, l1 * 65:l1 * 65 + 128],
                                 pt[:, 512:1024],
                                 start=(c == 0), stop=(c == NS - 1))

            for c in range(NS):
                if c == 0 and pre is not None:
                    pts[0] = pre
                else:
                    pts[c] = emit_scores(hp, qb, c)
                if c == 1 and carry_in is not None:
                    carry_in()
                if c >= 4:
                    emit_pv(c - 4)
                if filler is not None:
                    for fn in filler.get(c, ()):
                        fn()
            nxt_pt = preview_fn() if preview_fn is not None else None
            emit_pv(NS - 4)
            emit_pv(NS - 3)

            def carry_out():
                emit_pv(NS - 2)
                emit_pv(NS - 1)
                # fast-evict pv to SBUF, normalize off the critical path
                pvs = p_ev.tile([64, 1024], f32, tag="pvs", name="pvs")
                nc.vector.tensor_copy(out=pvs[:], in_=pv[0:64, :])
                srow = p1.tile([1, 1024], f32, tag="srow", name="srow")
                nc.vector.tensor_copy(out=srow[:], in_=pv[64:65, :])
                rsum = p1.tile([1, 1024], f32, tag="rsum", name="rsum")
                nc.vector.reciprocal_approx_fast(out=rsum[:], in_=srow[:])
                for h01 in range(2):
                    hsl = slice(h01 * 512, (h01 + 1) * 512)
                    rb = p1.tile([64, 512], f32, tag=f"rb{h01}", name="rb")
                    nc.gpsimd.partition_broadcast(rb[:], rsum[:, hsl])
                    lo = h01 * 64
                    nc.vector.tensor_tensor(out=ot[hp][lo:lo + 64, qsl],
                                            in0=pvs[0:64, hsl], in1=rb[:],
                                            op=MULT)

            def carry_final():
                emit_pv(NS - 2)
                emit_pv(NS - 1)
                # pipelined per-half: DVE row-copies first, Pool
                # broadcasts overlap, recip+mult read PV from PSUM
                srow = p1.tile([1, 1024], f32, tag="srow", name="srowf")
                for h01 in range(2):
                    hsl = slice(h01 * 512, (h01 + 1) * 512)
                    nc.vector.tensor_copy(out=srow[:, hsl],
                                          in_=pv[64:65, hsl])
                rbs = []
                for h01 in range(2):
                    hsl = slice(h01 * 512, (h01 + 1) * 512)
                    rbh = p1.tile([64, 512], f32, tag=f"rb{h01}", name="rbf")
                    nc.gpsimd.partition_broadcast(rbh[:], srow[:, hsl])
                    rbs.append(rbh)
                for h01 in range(2):
                    hsl = slice(h01 * 512, (h01 + 1) * 512)
                    nc.vector.reciprocal_approx_fast(out=rbs[h01][:],
                                                     in_=rbs[h01][:])
                    lo = h01 * 64
                    nc.vector.tensor_tensor(out=ot[hp][lo:lo + 64, qsl],
                                            in0=pv[0:64, hsl], in1=rbs[h01][:],
                                            op=MULT)

            return (carry_final if final else carry_out), nxt_pt

        def o01(qb, dt):
            """j=0,1 half of the output projection -> bf16 y01."""
            nsl = slice(qb * 512, (qb + 1) * 512)
            yps = psA.tile([128, 512], f32, tag="aux", name=f"o01_{dt}")
            for j in range(2):
                nc.tensor.matmul(
                    yps[:], wot[:, j * DIM + dt * 128:j * DIM + (dt + 1) * 128],
                    ot[j][:, nsl], start=(j == 0), stop=(j == 1))
            nc.vector.tensor_copy(out=y01[qb][:, dt * 512:(dt + 1) * 512],
                                  in_=yps[:])

        def o23(qb, dt):
            """j=2,3 half + add y01 + store."""
            nsl = slice(qb * 512, (qb + 1) * 512)
            yps = psA.tile([128, 512], f32, tag="aux", name=f"o23_{dt}")
            for j in (2, 3):
                nc.tensor.matmul(
                    yps[:], wot[:, j * DIM + dt * 128:j * DIM + (dt + 1) * 128],
                    ot[j][:, nsl], start=(j == 2), stop=(j == 3))
            ysb = p_y.tile([128, 512], bf16, tag="y", name="ysb")
            nc.vector.tensor_tensor(out=ysb[:], in0=yps[:],
                                    in1=y01[qb][:, dt * 512:(dt + 1) * 512],
                                    op=ADD)
            nc.sync.dma_start(out=yT[dt * 128:(dt + 1) * 128, nsl],
                              in_=ysb[:])

        def o23_tail(qb, dt):
            """Tail variant: stage into yfin (no buffer-reuse waits);
            odd dt evicts via Scalar + adds on Pool to unload DVE."""
            nsl = slice(qb * 512, (qb + 1) * 512)
            dsl = slice(dt * 512, (dt + 1) * 512)
            yps = psA.tile([128, 512], f32, tag="aux", name=f"of{dt}")
            for j in (2, 3):
                nc.tensor.matmul(
                    yps[:], wot[:, j * DIM + dt * 128:j * DIM + (dt + 1) * 128],
                    ot[j][:, nsl], start=(j == 2), stop=(j == 3))
            if dt % 2 == 0:
                nc.vector.tensor_tensor(out=yfin[:, dsl], in0=yps[:],
                                        in1=y01[qb][:, dsl], op=ADD)
            else:
                ycp = p_y.tile([128, 512], bf16, tag="y", name="ycp")
                nc.scalar.activation(ycp[:], yps[:], CPY)
                nc.gpsimd.tensor_tensor(out=yfin[:, dsl], in0=ycp[:],
                                        in1=y01[qb][:, dsl], op=ADD)
            nc.sync.dma_start(out=yT[dt * 128:(dt + 1) * 128, nsl],
                              in_=yfin[:, dsl])

        # ---- emission schedule -----------------------------------------
        qk_unit(0, "q", 0)
        qk_unit(0, "k", 0)

        V = lambda st: (lambda: v_unit(st))
        QK = lambda which, u: (lambda: qk_unit(0, which, u))
        O01 = lambda qb, dt: (lambda: o01(qb, dt))
        O23 = lambda qb, dt: (lambda: o23(qb, dt))
        fill_qb0 = {
            0: [V(0)], 1: [V(1), V(2)], 2: [V(3), QK("k", 1)], 3: [V(4)],
            4: [V(5)], 5: [V(6), QK("k", 2)], 6: [V(7)], 7: [V(8)],
            8: [V(9)], 9: [V(10), QK("k", 3)], 10: [V(11)],
            11: [V(12), QK("q", 1)], 12: [V(13)], 13: [V(14)], 14: [V(15)],
        }

        QKU = lambda hp, which, u: (lambda: qk_unit(hp, which, u))
        W2 = lambda: (lambda: warm(2))
        # filler schedule per (hp, qb): {slot: [thunks]}. Slots 0-3 are
        # the block-start hole (no PV matmuls until slot 4); next-hp
        # qk_units go there (slots 1/3) instead of trailing the block,
        # so the next block's scores reach the exp stream immediately.
        fills = {
            (0, 0): fill_qb0,
            (0, 1): {1: [QKU(1, "q", 0)], 2: [QK("q", 2)],
                     3: [QKU(1, "k", 0)]},
            (0, 2): {1: [QKU(1, "q", 1)], 2: [QK("q", 3)],
                     3: [QKU(1, "k", 1)]},
            (0, 3): {1: [QKU(1, "q", 2)], 3: [QKU(1, "k", 2)]},
            (1, 0): {1: [QKU(1, "q", 3)], 3: [QKU(1, "k", 3)]},
            (1, 1): {1: [QKU(2, "q", 0)], 3: [QKU(2, "k", 0)],
                     6: [O01(0, 0)], 8: [O01(0, 1)]},
            (1, 2): {0: [O01(0, 2)], 1: [QKU(2, "q", 1)],
                     3: [QKU(2, "k", 1)], 7: [O01(0, 3)], 9: [O01(0, 4)]},
            (1, 3): {0: [O01(0, 5)], 1: [QKU(2, "q", 2)],
                     3: [QKU(2, "k", 2)], 7: [O01(0, 6)], 9: [O01(0, 7)]},
            (2, 0): {0: [O01(1, 0)], 1: [QKU(2, "q", 3)],
                     3: [QKU(2, "k", 3)], 7: [O01(1, 1)], 9: [O01(1, 2)]},
            (2, 1): {0: [O01(1, 3)], 1: [QKU(3, "q", 0)],
                     3: [QKU(3, "k", 0)], 7: [O01(1, 4)], 9: [O01(1, 5)]},
            (2, 2): {0: [O01(1, 6)], 1: [QKU(3, "q", 1)],
                     3: [QKU(3, "k", 1)], 7: [O01(1, 7)], 9: [O01(2, 0)]},
            (2, 3): {0: [O01(2, 1)], 1: [QKU(3, "q", 2)],
                     3: [QKU(3, "k", 2)], 7: [O01(2, 2)], 9: [O01(2, 3)]},
            (3, 0): {0: [O01(2, 4)], 1: [QKU(3, "q", 3)],
                     3: [QKU(3, "k", 3)], 5: [O01(2, 5)], 6: [O01(2, 6)],
                     7: [O01(2, 7)], 8: [O01(3, 0)], 9: [O01(3, 1)],
                     10: [O01(3, 2)], 11: [O01(3, 3)]},
            (3, 1): {0: [O01(3, 4)], 1: [W2()], 3: [W2()],
                     5: [O01(3, 5)], 6: [O01(3, 6)], 7: [O01(3, 7)],
                     8: [O23(0, 0)], 9: [O23(0, 1)], 10: [O23(0, 2)],
                     11: [O23(0, 3)], 12: [O23(0, 4)]},
            (3, 2): {0: [W2()], 2: [W2()],
                     5: [O23(0, 5)], 6: [O23(0, 6)], 7: [O23(0, 7)],
                     8: [O23(1, 0)], 9: [O23(1, 1)], 10: [O23(1, 2)],
                     11: [O23(1, 3)], 12: [O23(1, 4)]},
            (3, 3): {0: [W2()], 2: [W2()],
                     5: [O23(1, 5)], 6: [O23(1, 6)], 7: [O23(1, 7)],
                     8: [O23(2, 0)], 9: [O23(2, 1)], 10: [O23(2, 2)],
                     11: [O23(2, 3)], 12: [O23(2, 4)]},
        }

        blocks = [(hp, qb) for hp in range(HP) for qb in range(NQ)]
        carry = None
        pre = None
        for i, (hp, qb) in enumerate(blocks):
            if qb == 0 and hp + 1 < HP:
                dma_qk_weights(hp + 1)
            final = (i == len(blocks) - 1)
            if final:
                preview_fn = None
            else:
                nhp, nqb = blocks[i + 1]
                preview_fn = (lambda h, q: (lambda: emit_scores(h, q, 0)))(
                    nhp, nqb)
            carry, pre = attention(hp, qb, fills.get((hp, qb)), carry,
                                   final=final, pre=pre,
                                   preview_fn=preview_fn)
        carry()
        # bridge PE over the final normalize (j=3 needs both head rows);
        # the last o23 units of qb2 are real work with no carry dep
        warm(8)
        warm(8)
        o23(NQ - 2, 5)
        o23(NQ - 2, 6)
        o23(NQ - 2, 7)
        for dt in range(KD):
            o23_tail(NQ - 1, dt)

    nc.finalize()
    return nc


def make_in_map(x_b, w_qkv, w_out, g):
    cols = slice(g * 512, (g + 1) * 512)
    b = ml_dtypes.bfloat16

    def qk_layout(w):
        # device reads [128, hp, k, 128] with contiguous per-hp slices
        return np.ascontiguousarray(
            w.reshape(KD, 128, HP, 128).transpose(1, 2, 0, 3)
             .reshape(128, HP * KD * 128).astype(b))

    wv_s = w_qkv[:, 2048:3072][:, cols]
    return {
        "xT": np.ascontiguousarray(x_b.T.astype(b)),
        "wq": qk_layout(w_qkv[:, 0:1024][:, cols]),
        "wk": qk_layout(w_qkv[:, 1024:2048][:, cols]),
        "wv": np.ascontiguousarray(
            wv_s.reshape(KD, 128, 512).transpose(1, 0, 2)
                .reshape(128, KD * 512).astype(b)),
        "wo": np.ascontiguousarray(w_out[cols, :].astype(b)),
    }


_NC_CACHE = {}


def _get_nc():
    if "nc" not in _NC_CACHE:
        _NC_CACHE["nc"] = build()
    return _NC_CACHE["nc"]


def kernel(x, w_qkv, w_out, b_out, trace=False):
    x = np.ascontiguousarray(np.asarray(x, dtype=np.float32))
    w_qkv = np.ascontiguousarray(np.asarray(w_qkv, dtype=np.float32))
    w_out = np.ascontiguousarray(np.asarray(w_out, dtype=np.float32))
    b_out = np.asarray(b_out, dtype=np.float32)

    nc = _get_nc()
    in_maps = [make_in_map(x[c // 2], w_qkv, w_out, c % 2) for c in range(8)]
    r = run_bass_kernel_spmd(nc, in_maps, list(range(8)), trace=trace)
    _NC_CACHE["exec_time_ns"] = r.exec_time_ns

    out = np.empty((B, N, DIM), np.float32)
    for b in range(B):
        out[b] = (r.results[2 * b]["yT"].astype(np.float32)
                  + r.results[2 * b + 1]["yT"].astype(np.float32)).T + b_out
    return out


# revision 38
# speedup vs baseline: 1.0046x; 1.0008x over previous
"""Multi-head self-attention TRN2 Bass kernel v7 (8 NeuronCores).

Sharding: core c -> batch b = c//2, head-group g = c%2 (8 of 16 heads).
Host sums the two partial output projections per batch.

v7 vs v6:
  - wq/wk/wv shipped host-transposed so weight DMAs are contiguous
    (256B descriptor lines -> 2KB), shrinking the preamble stall.
  - Output projection split j=0,1 (into bf16 y01 accumulators, run in
    hp1/hp2 exp-idle slots) + j=2,3 (+add) in hp3 / tail, so hp3 blocks
    drop to the exp floor and the tail only carries 16 matmuls.
  - hp3 block boundaries get post-block projection units (the exp
    stream drains ~2 chunks past each block end; PE needs queued work).
  - Final carry normalize pipelined per head-half across DVE/Pool,
    reading PV straight from PSUM (saves ~2.5us on the tail).
  - Tail adds alternate DVE and scalar-copy+Pool; output staged in a
    dedicated SBUF tile (no buffer-reuse waits on DMA completion).
"""
import numpy as np
from contextlib import ExitStack

import concourse.bass as bass
import concourse.mybir as mybir
import concourse.tile as tile
from concourse import bacc
from concourse.bass_utils import run_bass_kernel_spmd
import ml_dtypes

f32, f32r, bf16 = mybir.dt.float32, mybir.dt.float32r, mybir.dt.bfloat16
EXP = mybir.ActivationFunctionType.Exp
CPY = mybir.ActivationFunctionType.Copy
MULT = mybir.AluOpType.mult
ADD = mybir.AluOpType.add

B, N = 4, 2048
DIM = 1024
HL = 8
DH = 64
KD = DIM // 128
HP = HL // 2


def build(SEQ=2048):
    NS = SEQ // 128
    NQ = SEQ // 512

    nc = bacc.Bacc(None, target_bir_lowering=False, debug=False)
    xT = nc.declare_dram_parameter("xT", [DIM, SEQ], bf16, isOutput=False)
    wq = nc.declare_dram_parameter("wq", [128, HP * KD * 128], bf16,
                                   isOutput=False)
    wk = nc.declare_dram_parameter("wk", [128, HP * KD * 128], bf16,
                                   isOutput=False)
    wv = nc.declare_dram_parameter("wv", [128, KD * 512], bf16, isOutput=False)
    wo = nc.declare_dram_parameter("wo", [HL * DH, DIM], bf16, isOutput=False)
    yT = nc.declare_dram_parameter("yT", [DIM, SEQ], bf16, isOutput=True)

    with tile.TileContext(nc) as tc, ExitStack() as ctx:
        p1 = ctx.enter_context(tc.tile_pool(name="p1", bufs=1))
        p_pt = ctx.enter_context(tc.tile_pool(name="p_pt", bufs=6))
        p_w = ctx.enter_context(tc.tile_pool(name="p_w", bufs=2))
        p_y = ctx.enter_context(tc.tile_pool(name="p_y", bufs=4))
        p_ev = ctx.enter_context(tc.tile_pool(name="p_ev", bufs=2))
        psS = ctx.enter_context(tc.tile_pool(name="psS", bufs=2, space="PSUM"))
        psPV = ctx.enter_context(tc.tile_pool(name="psPV", bufs=1, space="PSUM"))
        psA = ctx.enter_context(tc.tile_pool(name="psA", bufs=2, space="PSUM"))

        # ---- persistent SBUF tiles -------------------------------------
        xt_all = p1.tile([128, KD * SEQ], bf16, tag="xt", name="xt")
        xt = [xt_all[:, k * SEQ:(k + 1) * SEQ] for k in range(KD)]
        wvt = p1.tile([128, KD * 512], bf16, tag="wvt", name="wvt")
        wot = p1.tile([128, HP * DIM], bf16, tag="wot", name="wot")
        q2 = [p1.tile([128, SEQ], bf16, tag=f"q2_{i}", name=f"q2_{i}")
              for i in range(2)]
        k2 = [p1.tile([128, SEQ], bf16, tag=f"k2_{i}", name=f"k2_{i}")
              for i in range(2)]
        v2 = [p1.tile([128, HL * 65 + 64], bf16, tag=f"v2_{st}", name=f"v2_{st}")
              for st in range(NS)]
        ot = [p1.tile([128, SEQ], bf16, tag=f"ot{j}", name=f"ot{j}")
              for j in range(HP)]
        # j=0,1 partial projections, one per query block (bf16)
        y01 = [p1.tile([128, KD * 512], bf16, tag=f"y01_{qb}", name=f"y01_{qb}")
               for qb in range(NQ)]
        yfin = p1.tile([128, KD * 512], bf16, tag="yfin", name="yfin")

        # ---- HAM warmup: dummy matmuls independent of any DMA ----------
        wsc = p1.tile([128, 512], bf16, tag="wsc", name="wsc")
        nc.gpsimd.memset(wsc[:], 0.0)
        for i in range(2):
            wps = psA.tile([128, 512], f32, tag="aux", name=f"hw{i}")
            for r in range(12):
                nc.tensor.matmul(wps[:], wsc[:, 0:128], wsc[:],
                                 start=(r == 0), stop=(r == 11))

        def warm(n, pool=None):
            """Discardable matmuls bridging PE over exp-stream restarts."""
            if pool is None:
                wps = psA.tile([128, 512], f32, tag="aux", name="warm")
            else:
                wps = pool.tile([128, 1024], f32, tag="s", name="warms")
            for r in range(n):
                nc.tensor.matmul(wps[:, 0:512], wsc[:, 0:128], wsc[:],
                                 start=(r == 0), stop=(r == n - 1))

        # ---- DMA issue (contiguous descriptors, consumption order) -----
        wt_q, wt_k = {}, {}

        def dma_qk_weights(hp):
            for which, wsrc, store in (("q", wq, wt_q), ("k", wk, wt_k)):
                t = p_w.tile([128, KD * 128], bf16, tag=f"w{which}",
                             name=f"w{which}{hp}")
                nc.sync.dma_start(
                    out=t[:], in_=wsrc[:, hp * KD * 128:(hp + 1) * KD * 128])
                store[hp] = t

        # hp0 weights split around x quarter 0: q-unit 0 needs wq+x only
        tq = p_w.tile([128, KD * 128], bf16, tag="wq", name="wq0")
        nc.sync.dma_start(out=tq[:], in_=wq[:, 0:KD * 128])
        wt_q[0] = tq
        for quarter in range(4):
            qsl2 = slice(quarter * (SEQ // 4), (quarter + 1) * (SEQ // 4))
            nc.sync.dma_start(
                out=xt_all[:].rearrange("p (k c) -> p k c", k=KD)[:, :, qsl2],
                in_=xT[:].rearrange("(k p) c -> p k c", k=KD)[:, :, qsl2])
            if quarter == 0:
                tk = p_w.tile([128, KD * 128], bf16, tag="wk", name="wk0")
                nc.sync.dma_start(out=tk[:], in_=wk[:, 0:KD * 128])
                wt_k[0] = tk
                nc.sync.dma_start(out=wvt[:], in_=wv[:])
        nc.sync.dma_start(
            out=wot[:].rearrange("p (j c) -> p j c", j=HP),
            in_=wo[:].rearrange("(j p) c -> p j c", j=HP))

        # ---- phase helpers ---------------------------------------------
        def qk_unit(hp, which, u):
            dst = (q2 if which == "q" else k2)[hp % 2]
            wt = (wt_q if which == "q" else wt_k)[hp]
            usl = slice(u * 512, (u + 1) * 512)
            pss = psA.tile([128, 512], f32, tag="aux", name=f"qk{which}{u}")
            for k in range(KD):
                nc.tensor.matmul(pss[:], wt[:, k * 128:(k + 1) * 128],
                                 xt[k][:, usl],
                                 start=(k == 0), stop=(k == KD - 1))
            nc.vector.tensor_copy(out=dst[:, usl], in_=pss[:])

        def v_unit(st):
            v2t = v2[st]
            v3 = v2t[:, 0:HL * 65].rearrange("p (h c) -> p h c", h=HL)
            nc.vector.memset(v3[:, :, 64:65], 1.0)
            nc.vector.memset(v2t[:, HL * 65:], 0.0)
            vps = psA.tile([128, HL * DH], f32, tag="aux", name=f"v{st}")
            for k in range(KD):
                nc.tensor.matmul(vps[:], xt[k][:, st * 128:(st + 1) * 128],
                                 wvt[:, k * 512:(k + 1) * 512],
                                 start=(k == 0), stop=(k == KD - 1))
            nc.vector.tensor_copy(
                out=v3[:, :, 0:64],
                in_=vps[:].rearrange("p (h d) -> p h d", h=HL))

        def emit_scores(hp, qb, c):
            """Score matmuls + exp for one key chunk; returns the pt tile."""
            q2t, k2t = q2[hp % 2], k2[hp % 2]
            qsl = slice(qb * 512, (qb + 1) * 512)
            csl = slice(c * 128, (c + 1) * 128)
            s = psS.tile([128, 1024], f32, tag="s", name="s")
            nc.tensor.matmul(s[:, 0:512], k2t[0:64, csl], q2t[0:64, qsl],
                             start=True, stop=True)
            nc.tensor.matmul(s[:, 512:1024], k2t[64:128, csl],
                             q2t[64:128, qsl], start=True, stop=True)
            pt = p_pt.tile([128, 1024], bf16, tag="pt", name="pt")
            nc.scalar.activation(pt[:], s[:], EXP, scale=0.125)
            return pt

        def attention(hp, qb, filler=None, carry_in=None, final=False,
                      pre=None, preview_fn=None):
            """Software-pipelined: PV lags its chunk by 4 slots; the
            previous block's last PVs + evict (carry_in) land at slot 1.
            Emission order per chunk: scores -> exp -> PV -> fillers.
            `pre` is this block's chunk-0 pt if the previous block
            previewed it; `preview_fn` emits the next block's chunk-0
            scores+exp before this block's trailing PVs so the exp
            stream never idles across the boundary."""
            qsl = slice(qb * 512, (qb + 1) * 512)
            l0, l1 = hp * 2, hp * 2 + 1
            pv = psPV.tile([128, 1024], f32, tag="pv", name="pv")
            pts = {}

            def emit_pv(c):
                pt = pts.pop(c)
                nc.tensor.matmul(pv[:, 0:512], v2[c][:, l0 * 65:l0 * 65 + 128],
                                 pt[:, 0:512],
                                 start=(c == 0), stop=(c == NS - 1))
                nc.tensor.matmul(pv[:, 512:1024],
                                 v2[c][:, l1 * 65:l1 * 65 + 128],
                                 pt[:, 512:1024],
                                 start=(c == 0), stop=(c == NS - 1))

            for c in range(NS):
                if c == 0 and pre is not None:
                    pts[0] = pre
                else:
                    pts[c] = emit_scores(hp, qb, c)
                if c == 1 and carry_in is not None:
                    carry_in()
                if c >= 4:
                    emit_pv(c - 4)
                if filler is not None:
                    for fn in filler.get(c, ()):
                        fn()
            nxt_pt = preview_fn() if preview_fn is not None else None
            emit_pv(NS - 4)
            emit_pv(NS - 3)

            def carry_out():
                emit_pv(NS - 2)
                emit_pv(NS - 1)
                # fast-evict pv to SBUF, normalize off the critical path
                pvs = p_ev.tile([64, 1024], f32, tag="pvs", name="pvs")
                nc.vector.tensor_copy(out=pvs[:], in_=pv[0:64, :])
                srow = p1.tile([1, 1024], f32, tag="srow", name="srow")
                nc.vector.tensor_copy(out=srow[:], in_=pv[64:65, :])
                rsum = p1.tile([1, 1024], f32, tag="rsum", name="rsum")
                nc.vector.reciprocal_approx_fast(out=rsum[:], in_=srow[:])
                for h01 in range(2):
                    hsl = slice(h01 * 512, (h01 + 1) * 512)
                    rb = p1.tile([64, 512], f32, tag=f"rb{h01}", name="rb")
                    nc.gpsimd.partition_broadcast(rb[:], rsum[:, hsl])
                    lo = h01 * 64
                    nc.vector.tensor_tensor(out=ot[hp][lo:lo + 64, qsl],
                                            in0=pvs[0:64, hsl], in1=rb[:],
                                            op=MULT)

            def carry_final():
                emit_pv(NS - 2)
                emit_pv(NS - 1)
                # pipelined per-half: DVE row-copies first, Pool
                # broadcasts overlap, recip+mult read PV from PSUM
                srow = p1.tile([1, 1024], f32, tag="srow", name="srowf")
                for h01 in range(2):
                    hsl = slice(h01 * 512, (h01 + 1) * 512)
                    nc.vector.tensor_copy(out=srow[:, hsl],
                                          in_=pv[64:65, hsl])
                rbs = []
                for h01 in range(2):
                    hsl = slice(h01 * 512, (h01 + 1) * 512)
                    rbh = p1.tile([64, 512], f32, tag=f"rb{h01}", name="rbf")
                    nc.gpsimd.partition_broadcast(rbh[:], srow[:, hsl])
                    rbs.append(rbh)
                for h01 in range(2):
                    hsl = slice(h01 * 512, (h01 + 1) * 512)
                    nc.vector.reciprocal_approx_fast(out=rbs[h01][:],
                                                     in_=rbs[h01][:])
                    lo = h01 * 64
                    nc.vector.tensor_tensor(out=ot[hp][lo:lo + 64, qsl],
                                            in0=pv[0:64, hsl], in1=rbs[h01][:],
                                            op=MULT)

            return (carry_final if final else carry_out), nxt_pt

        def o01(qb, dt):
            """j=0,1 half of the output projection -> bf16 y01."""
            nsl = slice(qb * 512, (qb + 1) * 512)
            yps = psA.tile([128, 512], f32, tag="aux", name=f"o01_{dt}")
            for j in range(2):
                nc.tensor.matmul(
                    yps[:], wot[:, j * DIM + dt * 128:j * DIM + (dt + 1) * 128],
                    ot[j][:, nsl], start=(j == 0), stop=(j == 1))
            nc.vector.tensor_copy(out=y01[qb][:, dt * 512:(dt + 1) * 512],
                                  in_=yps[:])

        def o23(qb, dt):
            """j=2,3 half + add y01 + store."""
            nsl = slice(qb * 512, (qb + 1) * 512)
            yps = psA.tile([128, 512], f32, tag="aux", name=f"o23_{dt}")
            for j in (2, 3):
                nc.tensor.matmul(
                    yps[:], wot[:, j * DIM + dt * 128:j * DIM + (dt + 1) * 128],
                    ot[j][:, nsl], start=(j == 2), stop=(j == 3))
            ysb = p_y.tile([128, 512], bf16, tag="y", name="ysb")
            nc.vector.tensor_tensor(out=ysb[:], in0=yps[:],
                                    in1=y01[qb][:, dt * 512:(dt + 1) * 512],
                                    op=ADD)
            nc.sync.dma_start(out=yT[dt * 128:(dt + 1) * 128, nsl],
                              in_=ysb[:])

        def o23_tail(qb, dt):
            """Tail variant: stage into yfin (no buffer-reuse waits);
            odd dt evicts via Scalar + adds on Pool to unload DVE."""
            nsl = slice(qb * 512, (qb + 1) * 512)
            dsl = slice(dt * 512, (dt + 1) * 512)
            yps = psA.tile([128, 512], f32, tag="aux", name=f"of{dt}")
            for j in (2, 3):
                nc.tensor.matmul(
                    yps[:], wot[:, j * DIM + dt * 128:j * DIM + (dt + 1) * 128],
                    ot[j][:, nsl], start=(j == 2), stop=(j == 3))
            if dt % 2 == 0:
                nc.vector.tensor_tensor(out=yfin[:, dsl], in0=yps[:],
                                        in1=y01[qb][:, dsl], op=ADD)
            else:
                ycp = p_y.tile([128, 512], bf16, tag="y", name="ycp")
                nc.scalar.activation(ycp[:], yps[:], CPY)
                nc.gpsimd.tensor_tensor(out=yfin[:, dsl], in0=ycp[:],
                                        in1=y01[qb][:, dsl], op=ADD)
            nc.sync.dma_start(out=yT[dt * 128:(dt + 1) * 128, nsl],
                              in_=yfin[:, dsl])

        # ---- emission schedule -----------------------------------------
        qk_unit(0, "q", 0)
        qk_unit(0, "k", 0)

        V = lambda st: (lambda: v_unit(st))
        QK = lambda which, u: (lambda: qk_unit(0, which, u))
        O01 = lambda qb, dt: (lambda: o01(qb, dt))
        O23 = lambda qb, dt: (lambda: o23(qb, dt))
        fill_qb0 = {
            0: [V(0)], 1: [V(1), V(2)], 2: [V(3), QK("k", 1)], 3: [V(4)],
            4: [V(5)], 5: [V(6), QK("k", 2)], 6: [V(7)], 7: [V(8)],
            8: [V(9)], 9: [V(10), QK("k", 3)], 10: [V(11)],
            11: [V(12), QK("q", 1)], 12: [V(13)], 13: [V(14)], 14: [V(15)],
        }

        QKU = lambda hp, which, u: (lambda: qk_unit(hp, which, u))
        W2 = lambda: (lambda: warm(2))
        # filler schedule per (hp, qb): {slot: [thunks]}. Slots 0-3 are
        # the block-start hole (no PV matmuls until slot 4); next-hp
        # qk_units go there (slots 1/3) instead of trailing the block,
        # so the next block's scores reach the exp stream immediately.
        fills = {
            (0, 0): fill_qb0,
            (0, 1): {1: [QKU(1, "q", 0)], 2: [QK("q", 2)],
                     3: [QKU(1, "k", 0)]},
            (0, 2): {1: [QKU(1, "q", 1)], 2: [QK("q", 3)],
                     3: [QKU(1, "k", 1)]},
            (0, 3): {1: [QKU(1, "q", 2)], 3: [QKU(1, "k", 2)]},
            (1, 0): {1: [QKU(1, "q", 3)], 3: [QKU(1, "k", 3)]},
            (1, 1): {1: [QKU(2, "q", 0)], 3: [QKU(2, "k", 0)],
                     6: [O01(0, 0)], 8: [O01(0, 1)]},
            (1, 2): {0: [W2()], 1: [QKU(2, "q", 1)], 3: [QKU(2, "k", 1)],
                     5: [O01(0, 2)], 7: [O01(0, 3)], 9: [O01(0, 4)]},
            (1, 3): {0: [O01(0, 5)], 1: [QKU(2, "q", 2)],
                     3: [QKU(2, "k", 2)], 7: [O01(0, 6)], 9: [O01(0, 7)]},
            (2, 0): {0: [O01(1, 0)], 1: [QKU(2, "q", 3)],
                     3: [QKU(2, "k", 3)], 7: [O01(1, 1)], 9: [O01(1, 2)]},
            (2, 1): {0: [O01(1, 3)], 1: [QKU(3, "q", 0)],
                     3: [QKU(3, "k", 0)], 7: [O01(1, 4)], 9: [O01(1, 5)]},
            (2, 2): {0: [O01(1, 6)], 1: [QKU(3, "q", 1)],
                     3: [QKU(3, "k", 1)], 7: [O01(1, 7)], 9: [O01(2, 0)]},
            (2, 3): {0: [O01(2, 1)], 1: [QKU(3, "q", 2)],
                     3: [QKU(3, "k", 2)], 7: [O01(2, 2)], 9: [O01(2, 3)]},
            (3, 0): {0: [O01(2, 4)], 1: [QKU(3, "q", 3)],
                     3: [QKU(3, "k", 3)], 5: [O01(2, 5)], 6: [O01(2, 6)],
                     7: [O01(2, 7)], 8: [O01(3, 0)], 9: [O01(3, 1)],
                     10: [O01(3, 2)], 11: [O01(3, 3)]},
            (3, 1): {0: [W2()], 1: [W2()], 3: [W2()],
                     5: [O01(3, 4)], 6: [O01(3, 5)], 7: [O01(3, 6)],
                     8: [O01(3, 7)], 9: [O23(0, 0)], 10: [O23(0, 1)],
                     11: [O23(0, 2)], 12: [O23(0, 3)], 13: [O23(0, 4)]},
            (3, 2): {0: [W2()], 2: [W2()],
                     5: [O23(0, 5)], 6: [O23(0, 6)], 7: [O23(0, 7)],
                     8: [O23(1, 0)], 9: [O23(1, 1)], 10: [O23(1, 2)],
                     11: [O23(1, 3)], 12: [O23(1, 4)]},
            (3, 3): {0: [W2()], 2: [W2()],
                     5: [O23(1, 5)], 6: [O23(1, 6)], 7: [O23(1, 7)],
                     8: [O23(2, 0)], 9: [O23(2, 1)], 10: [O23(2, 2)],
                     11: [O23(2, 3)], 12: [O23(2, 4)]},
        }

        blocks = [(hp, qb) for hp in range(HP) for qb in range(NQ)]
        carry = None
        pre = None
        for i, (hp, qb) in enumerate(blocks):
            if qb == 0 and hp + 1 < HP:
                dma_qk_weights(hp + 1)
            final = (i == len(blocks) - 1)
            if final:
                preview_fn = None
            else:
                nhp, nqb = blocks[i + 1]
                preview_fn = (lambda h, q: (lambda: emit_scores(h, q, 0)))(
                    nhp, nqb)
            carry, pre = attention(hp, qb, fills.get((hp, qb)), carry,
                                   final=final, pre=pre,
                                   preview_fn=preview_fn)
        carry()
        # bridge PE over the final normalize (j=3 needs both head rows);
        # the last o23 units of qb2 are real work with no carry dep
        warm(8)
        warm(8)
        o23(NQ - 2, 5)
        o23(NQ - 2, 6)
        o23(NQ - 2, 7)
        for dt in range(KD):
            o23_tail(NQ - 1, dt)

    nc.finalize()
    return nc


def make_in_map(x_b, w_qkv, w_out, g):
    cols = slice(g * 512, (g + 1) * 512)
    b = ml_dtypes.bfloat16

    def qk_layout(w):
        # device reads [128, hp, k, 128] with contiguous per-hp slices
        return np.ascontiguousarray(
            w.reshape(KD, 128, HP, 128).transpose(1, 2, 0, 3)
             .reshape(128, HP * KD * 128).astype(b))

    wv_s = w_qkv[:, 2048:3072][:, cols]
    return {
        "xT": np.ascontiguousarray(x_b.T.astype(b)),
        "wq": qk_layout(w_qkv[:, 0:1024][:, cols]),
        "wk": qk_layout(w_qkv[:, 1024:2048][:, cols]),
        "wv": np.ascontiguousarray(
            wv_s.reshape(KD, 128, 512).transpose(1, 0, 2)
                .reshape(128, KD * 512).astype(b)),
        "wo": np.ascontiguousarray(w_out[cols, :].astype(b)),
    }


_NC_CACHE = {}


def _get_nc():
    if "nc" not in _NC_CACHE:
        _NC_CACHE["nc"] = build()
    return _NC_CACHE["nc"]


def kernel(x, w_qkv, w_out, b_out, trace=False):
    x = np.ascontiguousarray(np.asarray(x, dtype=np.float32))
    w_qkv = np.ascontiguousarray(np.asarray(w_qkv, dtype=np.float32))
    w_out = np.ascontiguousarray(np.asarray(w_out, dtype=np.float32))
    b_out = np.asarray(b_out, dtype=np.float32)

    nc = _get_nc()
    in_maps = [make_in_map(x[c // 2], w_qkv, w_out, c % 2) for c in range(8)]
    r = run_bass_kernel_spmd(nc, in_maps, list(range(8)), trace=trace)
    _NC_CACHE["exec_time_ns"] = r.exec_time_ns

    out = np.empty((B, N, DIM), np.float32)
    for b in range(B):
        out[b] = (r.results[2 * b]["yT"].astype(np.float32)
                  + r.results[2 * b + 1]["yT"].astype(np.float32)).T + b_out
    return out
